# revision 2
# baseline (speedup 1.0000x reference)
"""CRNN (conv3x3 -> ReLU -> freq-maxpool -> GRU scan -> FC) on 8 Trainium2
NeuronCores — TIME-chunked sharding.

Each core processes ALL 64 batch items for a 160-step time window: its own
128 output steps plus 32 warm-up steps. The GRU forgets its initial state
at ~0.78/step, so a zero-started scan converges to ~4e-3 h-error after 32
steps (core 0 starts at t=0 with the exact h=0); host assembly keeps only
each core's 128 post-warm-up output columns. This cuts the serial scan from
1024 to 160 steps per core — the baseline's dominant cost (2.3 ms).

Layout: big SBUF tensors use (t, b)-interleaved columns j = 64*t + b, so a
scan step reads a contiguous [*, 64] tile and conv/xn stream N=512 bf16
matmuls (PE full rate at any N).

Per core (single woven pipeline):
  - conv: banded-weight bf16 matmuls (K=128 covers taps t-1/t, K=64 tap
    t+1) into [128 = 2f x 64c, 512] PSUM chunks; running tensor_max over
    the 32 f-pairs (DVE PSUM reads are the global bottleneck at ~2
    cyc/elem); fold + ReLU(+bias) -> feat (bf16-grade) in ufeat[64:128].
  - xn = W_ihn @ feat + b_in precomputed per chunk into xnBig.
  - GRU scan (fp32 state: bf16 state accumulates to ~3e-2 rel err):
    h_k = u_{k-1} + v_{k-1}; u_k = z*h_k (early, GPSIMD), v_k = (1-z)*n_k
    (late). rz and hn PSUMs take u and v as separate accumulating matmuls
    so only v is on the serial chain. u-mul and h-add run on GPSIMD to
    keep DVE free for woven conv drains.
  - WEAVE: conv/xn matmuls are emitted one 4-f-pair unit per scan step and
    their DVE drain closures are popped into the chain's ACT-wait gaps
    (sigma/tanh), overlapping the conv's DVE work with the latency-bound
    scan. rz/hn PSUM groups must not interleave within one bank (PSUM
    group tracking is bank-granular) — they use separate 1-buf pools.
  - FC transposed: per-step matmul with the h-tile as the STATIONARY
    operand writes psum[64b, 2] side by side into one [64, 320] PSUM bank
    -> single ACT copy -> one output DMA (the baseline's 128 strided
    output DMAs hid ~1 ms of DMA time). fc_b is added on the host.
"""

import contextlib
import numpy as np
import ml_dtypes

import concourse.bass as bass
import concourse.mybir as mybir
import concourse.tile as tile
from concourse import bacc
from concourse.bass_utils import run_bass_kernel_spmd

F32 = mybir.dt.float32
BF16 = mybir.dt.bfloat16
AF = mybir.ActivationFunctionType
OP = mybir.AluOpType

B, F, T = 64, 64, 1024
C = 64
H = 64
OUT = 2
NCORES = 8
NFP = F // 2          # 32 f-pairs
WARM = 32             # warm-up steps (cores 1..7)
CHUNK = T // NCORES   # 128 output steps per core
TW = CHUNK + WARM     # 176 scan steps per core
XW = TW + 2           # 178 x columns incl halo
JW = TW * B           # 11264 interleaved columns
NCH = JW // 512       # 22 conv/xn chunks
CLEAD = 3             # conv chunks emitted ahead of the scan
UH_GP = (True, True)  # (u-mul, h-add) on gpsimd?
MM_AFTER = False      # emit conv unit mms after the scan step's mms
SB_FP = 0             # f-pairs drained via ACT copy (ACT psum reads are slow)


def build_crnn(reps=1, phases=("conv", "xn", "scan", "fc"), debug=False,
               weave=True):
    nc = bacc.Bacc("TRN2", target_bir_lowering=False, debug=False)
    full = len(phases) == 4

    x_d = nc.declare_dram_parameter("x", [B, F, XW], F32, isOutput=False)
    convA_d = nc.declare_dram_parameter("convA", [128, NFP * 128], BF16, isOutput=False)
    convB_d = nc.declare_dram_parameter("convB", [64, NFP * 128], BF16, isOutput=False)
    cb_d = nc.declare_dram_parameter("conv_bias", [C, 1], F32, isOutput=False)
    wrz_d = nc.declare_dram_parameter("w_rz_lhsT", [128, 128], F32, isOutput=False)
    wn_d = nc.declare_dram_parameter("w_n_lhsT", [H, H], F32, isOutput=False)
    win_d = nc.declare_dram_parameter("w_in_lhsT", [C, H], F32, isOutput=False)
    brz_d = nc.declare_dram_parameter("b_rz", [128, 1], F32, isOutput=False)
    brzn_d = nc.declare_dram_parameter("b_rz_neg", [H, 1], F32, isOutput=False)
    bhn_d = nc.declare_dram_parameter("b_hn", [H, 1], F32, isOutput=False)
    bin_d = nc.declare_dram_parameter("b_in_row", [1, H], F32, isOutput=False)
    fcw_d = nc.declare_dram_parameter("fc_lhsT", [H, OUT], F32, isOutput=False)
    out_d = nc.declare_dram_parameter("out", [B, TW * OUT], F32, isOutput=True)
    if debug:
        feat_dbg = nc.declare_dram_parameter("feat_dbg", [C, JW], F32, isOutput=True)
        xn_dbg = nc.declare_dram_parameter("xn_dbg", [H, JW], BF16, isOutput=True)
        h_dbg = nc.declare_dram_parameter("h_dbg", [H, (TW + 1) * B], F32, isOutput=True)

    with tile.TileContext(nc) as tc:
        with (
            tc.tile_pool(name="persist", bufs=1) as persist,
            tc.tile_pool(name="stage", bufs=2) as stage,
            tc.tile_pool(name="work", bufs=2) as work,
            tc.tile_pool(name="sgp", bufs=4) as sgp,
            tc.tile_pool(name="scanw", bufs=2) as scanw,
            tc.tile_pool(name="pp_big", bufs=5, space="PSUM") as ppb,
            tc.tile_pool(name="pp_scan", bufs=1, space="PSUM") as pps,
            tc.tile_pool(name="pp_hn", bufs=1, space="PSUM") as pphn,
            tc.tile_pool(name="pp_fc", bufs=1, space="PSUM") as ppfc,
        ):
            convA = persist.tile([128, NFP * 128], BF16)
            convB = persist.tile([64, NFP * 128], BF16)
            cb = persist.tile([C, 1], F32)
            w_rz = persist.tile([128, 128], F32)
            w_n = persist.tile([H, H], F32)
            w_in_full = persist.tile([128, H], F32)
            w_in = w_in_full[64:128, :]
            b_rz = persist.tile([128, 1], F32)
            b_rz_neg = persist.tile([H, 1], F32)
            b_hn = persist.tile([H, 1], F32)
            b_in = persist.tile([1, H], F32)
            fc_w_full = persist.tile([128, OUT], F32)
            fc_w = fc_w_full[64:128, :]
            ones = persist.tile([1, 512], F32)
            # X2R: rows 0:64 hold x_win[ti] at col 64*(ti+1)+b (tap t-1 for
            # output col 64*t+b); rows 64:128 hold x_win[ti] at col 64*ti+b.
            X2R = persist.tile([128, (XW + 2) * B], BF16)
            # ufeat: rows 0:64 = u_{k-1} at block k, rows 64:128 = feat_k
            ufeat = persist.tile([128, (TW + 1) * B], F32)
            xnBig = persist.tile([H, JW], BF16)
            # base partition 64: pairs with rzt[64:128] in the u-multiply
            hB_full = persist.tile([128, (TW + 1) * B], F32)
            hB = hB_full[64:128, :]
            accs = [persist.tile([128, 512], BF16, name=f"acc{i}")
                    for i in range(NCH)]
            vts = [persist.tile([128, B], F32, name=f"vt{i}") for i in range(3)]

            nc.sync.dma_start(out=convA, in_=convA_d[:, :])
            nc.sync.dma_start(out=convB, in_=convB_d[:, :])
            nc.sync.dma_start(out=cb, in_=cb_d[:, :])
            nc.sync.dma_start(out=w_rz, in_=wrz_d[:, :])
            nc.sync.dma_start(out=w_n, in_=wn_d[:, :])
            nc.sync.dma_start(out=w_in, in_=win_d[:, :])
            nc.sync.dma_start(out=b_rz, in_=brz_d[:, :])
            nc.sync.dma_start(out=b_rz_neg, in_=brzn_d[:, :])
            nc.sync.dma_start(out=b_hn, in_=bhn_d[:, :])
            nc.sync.dma_start(out=b_in, in_=bin_d[:, :])
            nc.sync.dma_start(out=fc_w, in_=fcw_d[:, :])
            nc.vector.memset(ones, 1.0)
            nc.vector.memset(ufeat[0:64, 0:B], 0.0)   # u_{-1} = 0
            nc.vector.memset(hB[:, 0:B], 0.0)         # h_0 = 0
            if not full:
                nc.vector.memset(ufeat[:, :], 0.0)
                nc.vector.memset(hB[:, :], 0.0)
                nc.vector.memset(xnBig[:, :], 0.0)

            # ---- X2R staging (outside the rep loop, like the baseline) ----
            if "conv" in phases:
                for b in range(B):
                    stg = stage.tile([64, XW], F32, tag="stg", name="stg")
                    nc.sync.dma_start(out=stg, in_=x_d[b, :, :])
                    nc.vector.tensor_copy(
                        X2R[0:64, B + b : B + b + XW * B : B], stg)
                    nc.vector.tensor_copy(
                        X2R[64:128, b : b + XW * B : B], stg)

            conv_state = {}

            def conv_unit_mms(i, u, drains):
                """Emit unit u (f-pairs 4u..4u+3) of chunk i's matmuls; ACT
                copies go out immediately; DVE drain work is appended to
                `drains` as closures to be woven into scan-chain gaps."""
                if u == 0:
                    conv_state[i] = (
                        work.tile([128, 512], BF16, tag="cacc", name="cacc"),
                        (work.tile([128, 512], BF16, tag="caccg", name="caccg")
                         if SB_FP > 0 else None),
                    )
                acc, accg = conv_state[i]
                cs = slice(512 * i, 512 * (i + 1))
                for fp in range(4 * u, 4 * u + 4):
                    ps = ppb.tile([128, 512], F32, tag="cps", name="cps")
                    nc.tensor.matmul(
                        ps, convA[:, fp * 128 : (fp + 1) * 128],
                        X2R[:, B + 512 * i : B + 512 * i + 512],
                        start=True, stop=False)
                    nc.tensor.matmul(
                        ps, convB[:, fp * 128 : (fp + 1) * 128],
                        X2R[0:64, 3 * B + 512 * i : 3 * B + 512 * i + 512],
                        start=False, stop=True)
                    if fp < NFP - SB_FP:
                        if fp == 0:
                            drains.append(lambda a=acc, p=ps:
                                          nc.vector.tensor_copy(a, p))
                        else:
                            drains.append(lambda a=acc, p=ps:
                                          nc.vector.tensor_max(a, a, p))
                    else:
                        if fp == NFP - SB_FP:
                            nc.scalar.copy(accg, ps)
                        else:
                            sg = sgp.tile([128, 512], BF16, tag="sg", name="sg")
                            nc.scalar.copy(sg, ps)
                            drains.append(lambda a=accg, s=sg:
                                          nc.vector.tensor_max(a, a, s))
                if u == 7:
                    def tail(i=i, acc=acc, accg=accg, cs=cs):
                        if accg is not None:
                            nc.vector.tensor_max(acc, acc, accg)
                        mhi = work.tile([64, 512], BF16, tag="mhi", name="mhi")
                        nc.vector.tensor_copy(mhi, acc[64:128, :])
                        m2 = work.tile([64, 512], BF16, tag="m2", name="m2")
                        nc.vector.tensor_max(m2, acc[0:64, :], mhi)
                        nc.scalar.activation(
                            ufeat[64:128, cs], m2, AF.Relu, bias=cb)
                    drains.append(tail)
                    drains.append(lambda i=i: xn_chunk(i))
                    del conv_state[i]

            def conv_all():
                drains = []
                for i in range(NCH):
                    for u in range(8):
                        conv_unit_mms(i, u, drains)
                        while drains:
                            drains.pop(0)()

            def xn_chunk(i):
                ps = ppb.tile([128, 512], F32, tag="cps", name="xps")
                nc.tensor.matmul(
                    ps[0:64, :], w_in, ufeat[64:128, 512 * i : 512 * i + 512],
                    start=True, stop=False)
                nc.tensor.matmul(ps[0:64, :], b_in, ones,
                                 start=False, stop=True)
                nc.scalar.copy(xnBig[:, 512 * i : 512 * i + 512], ps[0:64, :])

            def xn_all():
                for i in range(NCH):
                    xn_chunk(i)

            def scan_step(k, fills=()):
                def fill(j):
                    if j < len(fills):
                        fills[j]()

                col = slice(k * B, (k + 1) * B)
                ncol = slice((k + 1) * B, (k + 2) * B)
                vprev = vts[k % 3]
                vnew = vts[(k + 1) % 3]
                psum_rz = pps.tile([128, B], F32, tag="sc", name="sc")
                psum_hn = pphn.tile([H, B], F32, tag="hn", name="hn")
                nc.tensor.matmul(psum_rz, w_rz, ufeat[:, col],
                                 start=True, stop=False)
                nc.tensor.matmul(psum_rz, w_rz, vprev[:, :],
                                 start=False, stop=True)
                nc.tensor.matmul(psum_hn, w_n, ufeat[0:64, col],
                                 start=True, stop=False)
                nc.tensor.matmul(psum_hn, w_n, vprev[0:64, :],
                                 start=False, stop=True)

                rzt = scanw.tile([128, B], F32, tag="rzt", name="rzt")
                nc.scalar.activation(rzt, psum_rz, AF.Sigmoid, bias=b_rz)
                zbt = scanw.tile([H, B], F32, tag="zbt", name="zbt")
                nc.scalar.activation(zbt, psum_rz[64:128, :], AF.Sigmoid,
                                     bias=b_rz_neg, scale=-1.0)
                # u_k = z * h_k (early; h_k written by previous step)
                (nc.gpsimd if UH_GP[0] else nc.vector).tensor_mul(
                    ufeat[0:64, ncol], rzt[64:128, :], hB[:, col])
                fill(0)  # runs on DVE during the sigma wait
                qt = scanw.tile([H, B], F32, tag="qt", name="qt")
                nc.vector.scalar_tensor_tensor(
                    out=qt, in0=psum_hn, scalar=b_hn, in1=rzt[0:64, :],
                    op0=OP.add, op1=OP.mult)
                q2t = scanw.tile([H, B], F32, tag="q2t", name="q2t")
                nc.vector.tensor_add(q2t, qt, xnBig[:, col])
                nt = scanw.tile([H, B], F32, tag="nt", name="nt")
                nc.scalar.activation(nt, q2t, AF.Tanh)
                fill(1)  # runs on DVE during the tanh wait
                nc.vector.tensor_mul(vnew[0:64, :], zbt, nt)
                (nc.gpsimd if UH_GP[1] else nc.vector).tensor_add(
                    hB[:, ncol], ufeat[0:64, ncol], vnew[0:64, :])
                fill(2)
                fill(3)
                for j in range(4, len(fills)):
                    fills[j]()

            def fc_all():
                psum_fc = ppfc.tile([B, TW * OUT], F32, tag="fc", name="fc")
                for k in range(TW):
                    nc.tensor.matmul(
                        psum_fc[:, OUT * k : OUT * (k + 1)],
                        hB[:, (k + 1) * B : (k + 2) * B], fc_w,
                        start=True, stop=True, skip_group_check=True)
                ob = work.tile([B, TW * OUT], F32, tag="ob", name="ob")
                nc.scalar.copy(ob, psum_fc)
                nc.sync.dma_start(out=out_d[:, :], in_=ob)

            rep_ctx = tc.For_i(0, reps, 1) if reps > 1 else contextlib.nullcontext()
            with rep_ctx:
                if full and weave:
                    # conv mms one unit per scan step; pure-DVE drain
                    # closures popped into the chain's ACT-wait gaps.
                    drains = []
                    for i in range(CLEAD):
                        for u in range(8):
                            conv_unit_mms(i, u, drains)
                    while drains:
                        drains.pop(0)()
                    for vt in vts:
                        nc.vector.memset(vt, 0.0)
                    pending = []
                    for k in range(TW):
                        i = k // 8 + CLEAD
                        if not MM_AFTER and i < NCH:
                            conv_unit_mms(i, k % 8, pending)
                        nf = min(4, len(pending))
                        fills, pending = pending[:nf], pending[nf:]
                        scan_step(k, fills)
                        if MM_AFTER and i < NCH:
                            conv_unit_mms(i, k % 8, pending)
                    for d in pending:
                        d()
                    fc_all()
                elif full:
                    conv_all()
                    for vt in vts:
                        nc.vector.memset(vt, 0.0)
                    for k in range(TW):
                        scan_step(k)
                    fc_all()
                else:
                    if "conv" in phases:
                        conv_all()
                    if "xn" in phases:
                        xn_all()
                    if "scan" in phases:
                        for vt in vts:
                            nc.vector.memset(vt, 0.0)
                        for k in range(TW):
                            scan_step(k)
                    if "fc" in phases:
                        fc_all()

    nc.finalize()
    return nc


def prep_weights(conv_w, conv_b, w_ih, w_hh, b_ih, b_hh, fc_w, fc_b):
    """Host-side rearrangement into device layouts (bf16 for matmul operands)."""
    bf = ml_dtypes.bfloat16
    conv_w = np.asarray(conv_w, np.float32)
    A = np.zeros((128, NFP * 128), np.float32)
    Bm = np.zeros((64, NFP * 128), np.float32)
    for fp in range(NFP):
        for fo in range(2):
            fout = 2 * fp + fo
            for fprime in range(max(0, fout - 1), min(64, fout + 2)):
                i = fprime - fout + 1
                cols = slice(fp * 128 + fo * 64, fp * 128 + fo * 64 + 64)
                A[fprime, cols] = conv_w[:, 0, i, 0]
                A[64 + fprime, cols] = conv_w[:, 0, i, 1]
                Bm[fprime, cols] = conv_w[:, 0, i, 2]
    w_ih = np.asarray(w_ih, np.float32)
    w_hh = np.asarray(w_hh, np.float32)
    b_ih = np.asarray(b_ih, np.float32)
    b_hh = np.asarray(b_hh, np.float32)
    return {
        "convA": A.astype(bf),
        "convB": Bm.astype(bf),
        "conv_bias": np.asarray(conv_b, np.float32).reshape(C, 1),
        "w_rz_lhsT": np.concatenate(
            [w_hh[0:128, :].T, w_ih[0:128, :].T], axis=0).astype(np.float32).copy(),
        "w_n_lhsT": w_hh[128:192, :].T.astype(np.float32).copy(),
        "w_in_lhsT": w_ih[128:192, :].T.astype(np.float32).copy(),
        "b_rz": (b_ih[0:128] + b_hh[0:128]).reshape(128, 1).astype(np.float32),
        "b_rz_neg": (-(b_ih[64:128] + b_hh[64:128])).reshape(H, 1).astype(np.float32),
        "b_hn": b_hh[128:192].reshape(H, 1).astype(np.float32),
        "b_in_row": b_ih[128:192].reshape(1, H).astype(np.float32),
        "fc_lhsT": np.asarray(fc_w, np.float32).T.copy(),
    }


def make_x_windows(x):
    """x [64,64,1024] fp32 -> list of 8 per-core windows [64,64,178]."""
    x = np.asarray(x, np.float32)
    xpad = np.zeros((B, F, T + 2), np.float32)
    xpad[:, :, 1 : T + 1] = x
    wins = []
    for c in range(NCORES):
        w0 = 0 if c == 0 else CHUNK * c - WARM
        wins.append(np.ascontiguousarray(xpad[:, :, w0 : w0 + XW]))
    return wins


def assemble_out(core_outs, fc_b):
    """core_outs: list of 8 arrays [64, TW*OUT] -> out [64, 2, 1024] fp32."""
    out = np.empty((B, OUT, T), np.float32)
    for c in range(NCORES):
        arr = np.asarray(core_outs[c]).reshape(B, TW, OUT)
        lo = 0 if c == 0 else WARM
        out[:, :, CHUNK * c : CHUNK * (c + 1)] = (
            arr[:, lo : lo + CHUNK, :].transpose(0, 2, 1))
    return out + np.asarray(fc_b, np.float32)[None, :, None]


_NC_CACHE = {}


def _get_nc():
    if "nc" not in _NC_CACHE:
        _NC_CACHE["nc"] = build_crnn()
    return _NC_CACHE["nc"]


def run(inputs, trace=False):
    wd = prep_weights(
        inputs["conv_w"], inputs["conv_b"], inputs["w_ih"], inputs["w_hh"],
        inputs["b_ih"], inputs["b_hh"], inputs["fc_w"], inputs["fc_b"],
    )
    wins = make_x_windows(inputs["x"])
    nc = _get_nc()
    in_maps = [dict(wd, x=wins[c]) for c in range(NCORES)]
    res = run_bass_kernel_spmd(nc, in_maps, list(range(NCORES)), trace=trace)
    out = assemble_out([res.results[c]["out"] for c in range(NCORES)],
                       inputs["fc_b"])
    return out, res


def kernel(**inputs) -> np.ndarray:
    out, _ = run(inputs, trace=False)
    return out


# revision 3
# speedup vs baseline: 1.0072x; 1.0072x over previous
"""CRNN (conv3x3 -> ReLU -> freq-maxpool -> GRU scan -> FC) on 8 Trainium2
NeuronCores — TIME-chunked sharding.

Each core processes ALL 64 batch items for a 160-step time window: its own
128 output steps plus 32 warm-up steps. The GRU forgets its initial state
at ~0.78/step, so a zero-started scan converges to ~4e-3 h-error after 32
steps (core 0 starts at t=0 with the exact h=0); host assembly keeps only
each core's 128 post-warm-up output columns. This cuts the serial scan from
1024 to 160 steps per core — the baseline's dominant cost (2.3 ms).

Layout: big SBUF tensors use (t, b)-interleaved columns j = 64*t + b, so a
scan step reads a contiguous [*, 64] tile and conv/xn stream N=512 bf16
matmuls (PE full rate at any N).

Per core (single woven pipeline):
  - conv: banded-weight bf16 matmuls (K=128 covers taps t-1/t, K=64 tap
    t+1) into [128 = 2f x 64c, 512] PSUM chunks; running tensor_max over
    the 32 f-pairs (DVE PSUM reads are the global bottleneck at ~2
    cyc/elem); fold + ReLU(+bias) -> feat (bf16-grade) in ufeat[64:128].
  - xn = W_ihn @ feat + b_in precomputed per chunk into xnBig.
  - GRU scan (fp32 state: bf16 state accumulates to ~3e-2 rel err):
    h_k = u_{k-1} + v_{k-1}; u_k = z*h_k (early, GPSIMD), v_k = (1-z)*n_k
    (late). rz and hn PSUMs take u and v as separate accumulating matmuls
    so only v is on the serial chain. u-mul and h-add run on GPSIMD to
    keep DVE free for woven conv drains.
  - WEAVE: conv/xn matmuls are emitted one 4-f-pair unit per scan step and
    their DVE drain closures are popped into the chain's ACT-wait gaps
    (sigma/tanh), overlapping the conv's DVE work with the latency-bound
    scan. rz/hn PSUM groups must not interleave within one bank (PSUM
    group tracking is bank-granular) — they use separate 1-buf pools.
  - FC transposed: per-step matmul with the h-tile as the STATIONARY
    operand writes psum[64b, 2] side by side into one [64, 320] PSUM bank
    -> single ACT copy -> one output DMA (the baseline's 128 strided
    output DMAs hid ~1 ms of DMA time). fc_b is added on the host.
"""

import contextlib
import numpy as np
import ml_dtypes

import concourse.bass as bass
import concourse.mybir as mybir
import concourse.tile as tile
from concourse import bacc
from concourse.bass_utils import run_bass_kernel_spmd

F32 = mybir.dt.float32
BF16 = mybir.dt.bfloat16
AF = mybir.ActivationFunctionType
OP = mybir.AluOpType

B, F, T = 64, 64, 1024
C = 64
H = 64
OUT = 2
NCORES = 8
NFP = F // 2          # 32 f-pairs
WARM = 32             # warm-up steps (cores 1..7)
CHUNK = T // NCORES   # 128 output steps per core
TW = CHUNK + WARM     # 176 scan steps per core
XW = TW + 2           # 178 x columns incl halo
JW = TW * B           # 11264 interleaved columns
NCH = JW // 512       # 22 conv/xn chunks
CLEAD = 3             # conv chunks emitted ahead of the scan
UH_GP = (True, True)  # (u-mul, h-add) on gpsimd?
MM_AFTER = False      # emit conv unit mms after the scan step's mms
SB_FP = 0             # f-pairs drained via ACT copy (ACT psum reads are slow)


def build_crnn(reps=1, phases=("conv", "xn", "scan", "fc"), debug=False,
               weave=True):
    nc = bacc.Bacc("TRN2", target_bir_lowering=False, debug=False)
    full = len(phases) == 4

    x_d = nc.declare_dram_parameter("x", [B, F, XW], F32, isOutput=False)
    convA_d = nc.declare_dram_parameter("convA", [128, NFP * 128], BF16, isOutput=False)
    convB_d = nc.declare_dram_parameter("convB", [64, NFP * 128], BF16, isOutput=False)
    cb_d = nc.declare_dram_parameter("conv_bias", [C, 1], F32, isOutput=False)
    wrz_d = nc.declare_dram_parameter("w_rz_lhsT", [128, 128], F32, isOutput=False)
    wn_d = nc.declare_dram_parameter("w_n_lhsT", [H, H], F32, isOutput=False)
    win_d = nc.declare_dram_parameter("w_in_lhsT", [C, H], F32, isOutput=False)
    brz_d = nc.declare_dram_parameter("b_rz", [128, 1], F32, isOutput=False)
    brzn_d = nc.declare_dram_parameter("b_rz_neg", [H, 1], F32, isOutput=False)
    bhn_d = nc.declare_dram_parameter("b_hn", [H, 1], F32, isOutput=False)
    bin_d = nc.declare_dram_parameter("b_in_row", [1, H], F32, isOutput=False)
    fcw_d = nc.declare_dram_parameter("fc_lhsT", [H, OUT], F32, isOutput=False)
    out_d = nc.declare_dram_parameter("out", [B, TW * OUT], F32, isOutput=True)
    if debug:
        feat_dbg = nc.declare_dram_parameter("feat_dbg", [C, JW], F32, isOutput=True)
        xn_dbg = nc.declare_dram_parameter("xn_dbg", [H, JW], BF16, isOutput=True)
        h_dbg = nc.declare_dram_parameter("h_dbg", [H, (TW + 1) * B], F32, isOutput=True)

    with tile.TileContext(nc) as tc:
        with (
            tc.tile_pool(name="persist", bufs=1) as persist,
            tc.tile_pool(name="stage", bufs=2) as stage,
            tc.tile_pool(name="work", bufs=2) as work,
            tc.tile_pool(name="sgp", bufs=4) as sgp,
            tc.tile_pool(name="scanw", bufs=2) as scanw,
            tc.tile_pool(name="pp_big", bufs=5, space="PSUM") as ppb,
            tc.tile_pool(name="pp_scan", bufs=1, space="PSUM") as pps,
            tc.tile_pool(name="pp_hn", bufs=1, space="PSUM") as pphn,
            tc.tile_pool(name="pp_fc", bufs=1, space="PSUM") as ppfc,
        ):
            convA = persist.tile([128, NFP * 128], BF16)
            convB = persist.tile([64, NFP * 128], BF16)
            cb = persist.tile([C, 1], F32)
            w_rz = persist.tile([128, 128], F32)
            w_n = persist.tile([H, H], F32)
            w_in_full = persist.tile([128, H], F32)
            w_in = w_in_full[64:128, :]
            b_rz = persist.tile([128, 1], F32)
            b_rz_neg = persist.tile([H, 1], F32)
            b_hn = persist.tile([H, 1], F32)
            b_in = persist.tile([1, H], F32)
            fc_w_full = persist.tile([128, OUT], F32)
            fc_w = fc_w_full[64:128, :]
            ones = persist.tile([1, 512], F32)
            # X2R: rows 0:64 hold x_win[ti] at col 64*(ti+1)+b (tap t-1 for
            # output col 64*t+b); rows 64:128 hold x_win[ti] at col 64*ti+b.
            X2R = persist.tile([128, (XW + 2) * B], BF16)
            # ufeat: rows 0:64 = u_{k-1} at block k, rows 64:128 = feat_k
            ufeat = persist.tile([128, (TW + 1) * B], F32)
            xnBig = persist.tile([H, JW], BF16)
            # base partition 64: pairs with rzt[64:128] in the u-multiply
            hB_full = persist.tile([128, (TW + 1) * B], F32)
            hB = hB_full[64:128, :]
            accs = [persist.tile([128, 512], BF16, name=f"acc{i}")
                    for i in range(NCH)]
            vts = [persist.tile([128, B], F32, name=f"vt{i}") for i in range(3)]

            nc.sync.dma_start(out=convA, in_=convA_d[:, :])
            nc.sync.dma_start(out=convB, in_=convB_d[:, :])
            nc.sync.dma_start(out=cb, in_=cb_d[:, :])
            nc.sync.dma_start(out=w_rz, in_=wrz_d[:, :])
            nc.sync.dma_start(out=w_n, in_=wn_d[:, :])
            nc.sync.dma_start(out=w_in, in_=win_d[:, :])
            nc.sync.dma_start(out=b_rz, in_=brz_d[:, :])
            nc.sync.dma_start(out=b_rz_neg, in_=brzn_d[:, :])
            nc.sync.dma_start(out=b_hn, in_=bhn_d[:, :])
            nc.sync.dma_start(out=b_in, in_=bin_d[:, :])
            nc.sync.dma_start(out=fc_w, in_=fcw_d[:, :])
            nc.vector.memset(ones, 1.0)
            nc.vector.memset(ufeat[0:64, 0:B], 0.0)   # u_{-1} = 0
            nc.vector.memset(hB[:, 0:B], 0.0)         # h_0 = 0
            if not full:
                nc.vector.memset(ufeat[:, :], 0.0)
                nc.vector.memset(hB[:, :], 0.0)
                nc.vector.memset(xnBig[:, :], 0.0)

            # ---- X2R staging (outside the rep loop, like the baseline) ----
            if "conv" in phases:
                for b in range(B):
                    stg = stage.tile([64, XW], F32, tag="stg", name="stg")
                    nc.sync.dma_start(out=stg, in_=x_d[b, :, :])
                    nc.vector.tensor_copy(
                        X2R[0:64, B + b : B + b + XW * B : B], stg)
                    nc.vector.tensor_copy(
                        X2R[64:128, b : b + XW * B : B], stg)

            conv_state = {}

            def conv_unit_mms(i, u, drains):
                """Emit unit u (f-pairs 4u..4u+3) of chunk i's matmuls; ACT
                copies go out immediately; DVE drain work is appended to
                `drains` as closures to be woven into scan-chain gaps."""
                if u == 0:
                    conv_state[i] = (
                        work.tile([128, 512], BF16, tag="cacc", name="cacc"),
                        (work.tile([128, 512], BF16, tag="caccg", name="caccg")
                         if SB_FP > 0 else None),
                    )
                acc, accg = conv_state[i]
                cs = slice(512 * i, 512 * (i + 1))
                for fp in range(4 * u, 4 * u + 4):
                    ps = ppb.tile([128, 512], F32, tag="cps", name="cps")
                    nc.tensor.matmul(
                        ps, convA[:, fp * 128 : (fp + 1) * 128],
                        X2R[:, B + 512 * i : B + 512 * i + 512],
                        start=True, stop=False)
                    nc.tensor.matmul(
                        ps, convB[:, fp * 128 : (fp + 1) * 128],
                        X2R[0:64, 3 * B + 512 * i : 3 * B + 512 * i + 512],
                        start=False, stop=True)
                    if fp < NFP - SB_FP:
                        if fp == 0:
                            drains.append(lambda a=acc, p=ps:
                                          nc.vector.tensor_copy(a, p))
                        else:
                            drains.append(lambda a=acc, p=ps:
                                          nc.vector.tensor_max(a, a, p))
                    else:
                        if fp == NFP - SB_FP:
                            nc.scalar.copy(accg, ps)
                        else:
                            sg = sgp.tile([128, 512], BF16, tag="sg", name="sg")
                            nc.scalar.copy(sg, ps)
                            drains.append(lambda a=accg, s=sg:
                                          nc.vector.tensor_max(a, a, s))
                if u == 7:
                    def tail(i=i, acc=acc, accg=accg, cs=cs):
                        if accg is not None:
                            nc.vector.tensor_max(acc, acc, accg)
                        mhi = work.tile([64, 512], BF16, tag="mhi", name="mhi")
                        nc.vector.tensor_copy(mhi, acc[64:128, :])
                        m2 = work.tile([64, 512], BF16, tag="m2", name="m2")
                        nc.vector.tensor_max(m2, acc[0:64, :], mhi)
                        nc.scalar.activation(
                            ufeat[64:128, cs], m2, AF.Relu, bias=cb)
                    drains.append(tail)
                    drains.append(lambda i=i: xn_chunk(i))
                    del conv_state[i]

            def conv_all():
                drains = []
                for i in range(NCH):
                    for u in range(8):
                        conv_unit_mms(i, u, drains)
                        while drains:
                            drains.pop(0)()

            def xn_chunk(i):
                ps = ppb.tile([128, 512], F32, tag="cps", name="xps")
                nc.tensor.matmul(
                    ps[0:64, :], w_in, ufeat[64:128, 512 * i : 512 * i + 512],
                    start=True, stop=False)
                nc.tensor.matmul(ps[0:64, :], b_in, ones,
                                 start=False, stop=True)
                nc.scalar.copy(xnBig[:, 512 * i : 512 * i + 512], ps[0:64, :])

            def xn_all():
                for i in range(NCH):
                    xn_chunk(i)

            def scan_step(k, fills=()):
                def fill(j):
                    if j < len(fills):
                        fills[j]()

                col = slice(k * B, (k + 1) * B)
                ncol = slice((k + 1) * B, (k + 2) * B)
                vprev = vts[k % 3]
                vnew = vts[(k + 1) % 3]
                psum_rz = pps.tile([128, B], F32, tag="sc", name="sc")
                psum_hn = pphn.tile([H, B], F32, tag="hn", name="hn")
                nc.tensor.matmul(psum_rz, w_rz, ufeat[:, col],
                                 start=True, stop=False)
                nc.tensor.matmul(psum_rz, w_rz, vprev[:, :],
                                 start=False, stop=True)
                nc.tensor.matmul(psum_hn, w_n, ufeat[0:64, col],
                                 start=True, stop=False)
                nc.tensor.matmul(psum_hn, w_n, vprev[0:64, :],
                                 start=False, stop=True)

                rzt = scanw.tile([128, B], F32, tag="rzt", name="rzt")
                nc.scalar.activation(rzt, psum_rz, AF.Sigmoid, bias=b_rz)
                zbt = scanw.tile([H, B], F32, tag="zbt", name="zbt")
                nc.scalar.activation(zbt, psum_rz[64:128, :], AF.Sigmoid,
                                     bias=b_rz_neg, scale=-1.0)
                # u_k = z * h_k (early; h_k written by previous step)
                (nc.gpsimd if UH_GP[0] else nc.vector).tensor_mul(
                    ufeat[0:64, ncol], rzt[64:128, :], hB[:, col])
                fill(0)  # runs on DVE during the sigma wait
                qt = scanw.tile([H, B], F32, tag="qt", name="qt")
                nc.vector.scalar_tensor_tensor(
                    out=qt, in0=psum_hn, scalar=b_hn, in1=rzt[0:64, :],
                    op0=OP.add, op1=OP.mult)
                q2t = scanw.tile([H, B], F32, tag="q2t", name="q2t")
                nc.vector.tensor_add(q2t, qt, xnBig[:, col])
                nt = scanw.tile([H, B], F32, tag="nt", name="nt")
                nc.scalar.activation(nt, q2t, AF.Tanh)
                fill(1)  # runs on DVE during the tanh wait
                nc.vector.tensor_mul(vnew[0:64, :], zbt, nt)
                (nc.gpsimd if UH_GP[1] else nc.vector).tensor_add(
                    hB[:, ncol], ufeat[0:64, ncol], vnew[0:64, :])
                fill(2)
                fill(3)
                for j in range(4, len(fills)):
                    fills[j]()

            def fc_all():
                psum_fc = ppfc.tile([B, TW * OUT], F32, tag="fc", name="fc")
                for k in range(TW):
                    nc.tensor.matmul(
                        psum_fc[:, OUT * k : OUT * (k + 1)],
                        hB[:, (k + 1) * B : (k + 2) * B], fc_w,
                        start=True, stop=True, skip_group_check=True)
                ob = work.tile([B, TW * OUT], F32, tag="ob", name="ob")
                nc.scalar.copy(ob, psum_fc)
                nc.sync.dma_start(out=out_d[:, :], in_=ob)

            rep_ctx = tc.For_i(0, reps, 1) if reps > 1 else contextlib.nullcontext()
            with rep_ctx:
                if full and weave:
                    # conv mms one unit per scan step; pure-DVE drain
                    # closures popped into the chain's ACT-wait gaps.
                    drains = []
                    for i in range(CLEAD):
                        for u in range(8):
                            conv_unit_mms(i, u, drains)
                    while drains:
                        drains.pop(0)()
                    for vt in vts:
                        nc.vector.memset(vt, 0.0)
                    pending = []
                    psum_fc = ppfc.tile([B, TW * OUT], F32, tag="fc", name="fc")
                    for k in range(TW):
                        i = k // 8 + CLEAD
                        if not MM_AFTER and i < NCH:
                            conv_unit_mms(i, k % 8, pending)
                        nf = min(5, len(pending))
                        fills, pending = pending[:nf], pending[nf:]
                        scan_step(k, fills)
                        if k >= 2:
                            nc.tensor.matmul(
                                psum_fc[:, OUT * (k - 2) : OUT * (k - 1)],
                                hB[:, (k - 1) * B : k * B], fc_w,
                                start=True, stop=True, skip_group_check=True)
                        if MM_AFTER and i < NCH:
                            conv_unit_mms(i, k % 8, pending)
                    for d in pending:
                        d()
                    for k in (TW - 2, TW - 1):
                        nc.tensor.matmul(
                            psum_fc[:, OUT * k : OUT * (k + 1)],
                            hB[:, (k + 1) * B : (k + 2) * B], fc_w,
                            start=True, stop=True, skip_group_check=True)
                    ob = work.tile([B, TW * OUT], F32, tag="ob", name="ob")
                    nc.scalar.copy(ob, psum_fc)
                    nc.sync.dma_start(out=out_d[:, :], in_=ob)
                elif full:
                    conv_all()
                    for vt in vts:
                        nc.vector.memset(vt, 0.0)
                    for k in range(TW):
                        scan_step(k)
                    fc_all()
                else:
                    if "conv" in phases:
                        conv_all()
                    if "xn" in phases:
                        xn_all()
                    if "scan" in phases:
                        for vt in vts:
                            nc.vector.memset(vt, 0.0)
                        for k in range(TW):
                            scan_step(k)
                    if "fc" in phases:
                        fc_all()

    nc.finalize()
    return nc


def prep_weights(conv_w, conv_b, w_ih, w_hh, b_ih, b_hh, fc_w, fc_b):
    """Host-side rearrangement into device layouts (bf16 for matmul operands)."""
    bf = ml_dtypes.bfloat16
    conv_w = np.asarray(conv_w, np.float32)
    A = np.zeros((128, NFP * 128), np.float32)
    Bm = np.zeros((64, NFP * 128), np.float32)
    for fp in range(NFP):
        for fo in range(2):
            fout = 2 * fp + fo
            for fprime in range(max(0, fout - 1), min(64, fout + 2)):
                i = fprime - fout + 1
                cols = slice(fp * 128 + fo * 64, fp * 128 + fo * 64 + 64)
                A[fprime, cols] = conv_w[:, 0, i, 0]
                A[64 + fprime, cols] = conv_w[:, 0, i, 1]
                Bm[fprime, cols] = conv_w[:, 0, i, 2]
    w_ih = np.asarray(w_ih, np.float32)
    w_hh = np.asarray(w_hh, np.float32)
    b_ih = np.asarray(b_ih, np.float32)
    b_hh = np.asarray(b_hh, np.float32)
    return {
        "convA": A.astype(bf),
        "convB": Bm.astype(bf),
        "conv_bias": np.asarray(conv_b, np.float32).reshape(C, 1),
        "w_rz_lhsT": np.concatenate(
            [w_hh[0:128, :].T, w_ih[0:128, :].T], axis=0).astype(np.float32).copy(),
        "w_n_lhsT": w_hh[128:192, :].T.astype(np.float32).copy(),
        "w_in_lhsT": w_ih[128:192, :].T.astype(np.float32).copy(),
        "b_rz": (b_ih[0:128] + b_hh[0:128]).reshape(128, 1).astype(np.float32),
        "b_rz_neg": (-(b_ih[64:128] + b_hh[64:128])).reshape(H, 1).astype(np.float32),
        "b_hn": b_hh[128:192].reshape(H, 1).astype(np.float32),
        "b_in_row": b_ih[128:192].reshape(1, H).astype(np.float32),
        "fc_lhsT": np.asarray(fc_w, np.float32).T.copy(),
    }


def make_x_windows(x):
    """x [64,64,1024] fp32 -> list of 8 per-core windows [64,64,178]."""
    x = np.asarray(x, np.float32)
    xpad = np.zeros((B, F, T + 2), np.float32)
    xpad[:, :, 1 : T + 1] = x
    wins = []
    for c in range(NCORES):
        w0 = 0 if c == 0 else CHUNK * c - WARM
        wins.append(np.ascontiguousarray(xpad[:, :, w0 : w0 + XW]))
    return wins


def assemble_out(core_outs, fc_b):
    """core_outs: list of 8 arrays [64, TW*OUT] -> out [64, 2, 1024] fp32."""
    out = np.empty((B, OUT, T), np.float32)
    for c in range(NCORES):
        arr = np.asarray(core_outs[c]).reshape(B, TW, OUT)
        lo = 0 if c == 0 else WARM
        out[:, :, CHUNK * c : CHUNK * (c + 1)] = (
            arr[:, lo : lo + CHUNK, :].transpose(0, 2, 1))
    return out + np.asarray(fc_b, np.float32)[None, :, None]


_NC_CACHE = {}


def _get_nc():
    if "nc" not in _NC_CACHE:
        _NC_CACHE["nc"] = build_crnn()
    return _NC_CACHE["nc"]


def run(inputs, trace=False):
    wd = prep_weights(
        inputs["conv_w"], inputs["conv_b"], inputs["w_ih"], inputs["w_hh"],
        inputs["b_ih"], inputs["b_hh"], inputs["fc_w"], inputs["fc_b"],
    )
    wins = make_x_windows(inputs["x"])
    nc = _get_nc()
    in_maps = [dict(wd, x=wins[c]) for c in range(NCORES)]
    res = run_bass_kernel_spmd(nc, in_maps, list(range(NCORES)), trace=trace)
    out = assemble_out([res.results[c]["out"] for c in range(NCORES)],
                       inputs["fc_b"])
    return out, res


def kernel(**inputs) -> np.ndarray:
    out, _ = run(inputs, trace=False)
    return out
